# revision 20
# baseline (speedup 1.0000x reference)
"""Multi-head attention (B=2, H=16, S=4096, D=64, fp16) on 8 TRN2 NeuronCores.

Sharding: the 32 (b, h) head-slices are split 4-per-core (data/head
parallel, no cross-core communication). Each core runs a flash-attention
style kernel over its 4 heads.

Per-head algorithm (transposed-scores layout, no on-device transposes):
  - Host pre-lays-out inputs: QT[d, s] = Q^T, KTp[d, j*128+p] = K[p*32+j, d]
    (a t-permutation that makes the V load contiguous), and VA = [V | 1]
    (ones column => the PV matmul also produces the softmax denominator).
    QT/KT are loaded twice (partitions 0-63 and 64-127) so score matmuls can
    be row-packed onto both halves of the PE array (concurrent execution).
  - scores^T tile [t=128, s=512] = KTp_tile.T @ QT_tile   (PE, K=64)
  - P^T = exp(scale * scores^T)  fp32->fp16. The exp work is split between
    two engines, both reading the PSUM scores directly:
      * ACT groups: nc.scalar.activation(Exp)             (1 elem/cyc/lane)
      * DVE groups: one fused tensor_scalar computing
           i16 = round(score * (scale*log2e*2^10) + (15*2^10 - C))
        whose int16 bits, reinterpreted as fp16, are 2^(score*scale*log2e)
        with mantissa-linear (Schraudolph) interpolation: ~2% RMS rel err
        on those tiles, which dilutes to ~1e-2 on the final output --
        inside the 2e-2 accuracy budget.
  - PV (swapped operands): for each 128-wide query block sb,
        out[sb, 0:65] += P^T[t, sb].T @ VA[t, 0:65]
    i.e. the P^T tile is the *stationary* operand (128x128 fp16 loads get
    FWL's 2x weight-load rate) and VA streams only 65 columns. Measured
    45.5 ns/tile vs 66 ns/tile for the N=512 streaming formulation, and the
    accumulator is already in [s, d] layout -- no transposes needed.
    Column 64 of the accumulator is the softmax denominator.
  - fixup per 512-wide chunk: reciprocal of col 64, broadcast-multiply
    cols 0-63 (DVE, straight from PSUM), DMA out [s, d].

The emission runs a one-window software pipeline: while window w's scores
stream through PE->{ACT,DVE}, the PV matmuls consume window w-1's probs
(already in SBUF). Softmax skips max-subtraction: scores ~ N(0,1) after
scaling (measured |score*scale| < 6), so fp32/fp16 exp are safe.
"""

from contextlib import ExitStack

import numpy as np

import concourse.bass as bass
import concourse.tile as tile
from concourse import bacc, mybir
from concourse.bass_utils import run_bass_kernel_spmd

B, H, S, D = 2, 16, 4096, 64
N_CORES = 8
HPC = (B * H) // N_CORES  # heads per core
SCALE = float(D) ** -0.5
SQ = 512  # s-chunk width (one PSUM bank of fp32)
G = 2  # t-tiles (PSUM banks) per exp group
WIN = 2 * SQ  # s-window: scores/exp pipelining granularity

ROWPACK_SCORES = True  # tile_position row-packed scores matmuls
WARMUP = True  # HAM warmup matmul block

# Schraudolph exp2-in-fp16-bits constants (DVE exp path)
LOG2E = 1.4426950408889634
EXP_A = float(SCALE * LOG2E * 1024.0)
EXP_C = 60.0  # centering constant, tuned for min output L2 error
EXP_B = float(15 * 1024 - EXP_C)
# exp engine split: chunk 0 -> ACT, chunk 1 -> DVE (independent pacing
# lanes); group indices listed here run chunk 1 on ACT too (balance knob).
ACT_EXTRA = frozenset({15})


def attention_body(tc, qt, kt, va, o, heads, s, d):
    """Emit the per-core attention program.

    qt: [heads, d, s] fp16   Q^T per head
    kt: [heads, d, s] fp16   K^T per head, t-permuted (col j*128+p = row p*(s//128)+j)
    va: [heads, s, d+1] fp16 V with ones column, partition-major
    o:  [heads, s, d] fp16   output
    """
    nc = tc.nc
    f32 = mybir.dt.float32
    f16 = mybir.dt.float16
    i16 = mybir.dt.int16
    nt = s // 128  # number of 128-row t tiles
    nwin = s // WIN  # s windows per head
    nsb = SQ // 128  # 128-wide query blocks per chunk

    groups = []
    t0 = 0
    while t0 < nt:
        gs = min(G, nt - t0)
        groups.append((t0, gs))
        t0 += gs

    with ExitStack() as ctx:
        qk_pool = ctx.enter_context(tc.tile_pool(name="qk", bufs=2))
        v_pool = ctx.enter_context(tc.tile_pool(name="v", bufs=2))
        # probs live from their exp (window w) until consumed by PV during
        # window w+1: ~2 windows of groups in flight.
        # probs live ~PV_LAG+1 groups (exp at group g, consumed by the PV
        # slice interleaved after group g+PV_LAG): small rotating pool.
        p_pool = ctx.enter_context(tc.tile_pool(name="p", bufs=12))
        ps_pool = ctx.enter_context(tc.tile_pool(name="ps", bufs=3, space="PSUM"))
        po_pool = ctx.enter_context(tc.tile_pool(name="po", bufs=2, space="PSUM"))
        fix_pool = ctx.enter_context(tc.tile_pool(name="fix", bufs=2))
        const_pool = ctx.enter_context(tc.tile_pool(name="const", bufs=1))

        if WARMUP:
            # ~16 back-to-back matmuls trip the HAM activity window early so
            # the PE runs at 2.4 GHz instead of staying clock-gated at 1.2.
            warm_src = const_pool.tile([d + 1, SQ], f16)
            nc.vector.memset(warm_src, 1.0)
            warm_w = const_pool.tile([d + 1, d + 1], f16)
            nc.vector.memset(warm_w, 1.0)
            warm_ps = ps_pool.tile([128, G, SQ], f32, tag="ps")
            for i in range(16):
                nc.tensor.matmul(
                    warm_ps[: d + 1, 0, :],
                    lhsT=warm_w,
                    rhs=warm_src,
                    start=True,
                    stop=True,
                )

        # Per-head SBUF tiles, fetched lazily at head boundaries.
        head_tiles = {}

        def load_head(h):
            # Chunked loads ordered by first use so the first window's scores
            # only wait on the leading slices (Tile tracks byte-range deps).
            nck = 4
            cs = s // nck
            qt_sb = qk_pool.tile([128 if ROWPACK_SCORES else 64, s], f16, tag="qt")
            kt_sb = qk_pool.tile([128 if ROWPACK_SCORES else 64, s], f16, tag="kt")
            va_sb = v_pool.tile([128, nt, d + 1], f16, tag="va")
            va_src = va[h].rearrange("(p i) e -> p i e", p=128)
            rows = [0, 64] if ROWPACK_SCORES else [0]
            ick = nt // nck

            def kt_chunk(k):
                sl = slice(k * cs, (k + 1) * cs)
                for rp in rows:
                    nc.sync.dma_start(out=kt_sb[rp : rp + 64, sl], in_=kt[h][:, sl])

            def qt_chunk(k):
                sl = slice(k * cs, (k + 1) * cs)
                for rp in rows:
                    nc.sync.dma_start(out=qt_sb[rp : rp + 64, sl], in_=qt[h][:, sl])

            # kt chunk 0 + qt chunk 0 unblock the first window's scores; va is
            # first needed a window later; qt tails are needed last.
            kt_chunk(0)
            qt_chunk(0)
            for k in range(1, nck):
                kt_chunk(k)
            for k in range(nck):
                nc.sync.dma_start(
                    out=va_sb[:, k * ick : (k + 1) * ick, :],
                    in_=va_src[:, k * ick : (k + 1) * ick, :],
                )
            for k in range(1, nck):
                qt_chunk(k)
            head_tiles[h] = (qt_sb, kt_sb, va_sb)

        score_mm_count = [0]  # global parity so consecutive mms always row-pair

        def emit_score_group(h, w, gi):
            """Scores (both chunks, fully row-paired) + exp for group gi."""
            qt_sb, kt_sb, _ = head_tiles[h]
            w0 = w * WIN
            t0, gs = groups[gi]
            pss = []
            # all 2*gs score matmuls adjacent on the PE queue with alternating
            # row parity -> every consecutive pair runs concurrently
            for c in (0, 1):
                ps = ps_pool.tile([128, G, SQ], f32, tag="ps", name=f"ps{c}")
                pss.append(ps)
            for c in (0, 1):
                for g in range(gs):
                    t = t0 + g
                    rp = 64 * (score_mm_count[0] % 2) if ROWPACK_SCORES else 0
                    score_mm_count[0] += 1
                    nc.tensor.matmul(
                        pss[c][:, g, :],
                        lhsT=kt_sb[rp : rp + 64, t * 128 : (t + 1) * 128],
                        rhs=qt_sb[rp : rp + 64, w0 + c * SQ : w0 + (c + 1) * SQ],
                        start=True,
                        stop=True,
                        tile_position=(rp, 0) if ROWPACK_SCORES else None,
                    )
            pts = []
            for c in (0, 1):
                pt = p_pool.tile([128, G, SQ], f16, tag="pt", name=f"pt{c}")
                if c == 1 and gi not in ACT_EXTRA:
                    nc.vector.tensor_scalar(
                        pt[:, :gs, :].bitcast(i16),
                        pss[c][:, :gs, :],
                        EXP_A,
                        EXP_B,
                        mybir.AluOpType.mult,
                        mybir.AluOpType.add,
                    )
                else:
                    nc.scalar.activation(
                        pt[:, :gs, :],
                        pss[c][:, :gs, :],
                        mybir.ActivationFunctionType.Exp,
                        scale=SCALE,
                    )
                pts.append(pt)
            return pts

        ngrp = len(groups)
        PV_LAG = 2  # groups between a group's exp and its PV slice

        def pv_slices(h, w, win_pts, pvs):
            """Yields once per group after emitting that group's PV matmuls.

            The t-accumulation is ordered by group so PV of window w runs
            *inside* window w, lagged PV_LAG groups behind the exps: the
            software pipeline is ~2 groups deep instead of a full window
            (cheap fill/drain, small probs pool), and the PE interleaves
            scores + PV continuously (HAM stays at full clock)."""
            _, _, va_sb = head_tiles[h]
            for g in range(ngrp):
                t0, gs = groups[g]
                pts = win_pts[g]  # filled by the caller before this slice runs
                for c in (0, 1):
                    for sb in range(nsb):
                        for gg in range(gs):
                            t = t0 + gg
                            # start=True clears has_written for the WHOLE
                            # bank, so only the bank's first mm may set it;
                            # the other sb-chains' first writes land on
                            # cleared bits and overwrite (then accumulate).
                            nc.tensor.matmul(
                                pvs[c][:, sb, :],
                                lhsT=pts[c][:, gg, sb * 128 : (sb + 1) * 128],
                                rhs=va_sb[:, t, :],
                                start=(t == 0 and sb == 0),
                                stop=(t == nt - 1 and sb == nsb - 1),
                                skip_group_check=True,
                            )
                yield

        def fixup(h, w, pvs, c):
            pv = pvs[c]
            rec = fix_pool.tile([128, nsb], f32, tag=f"rec{c}")
            nc.vector.reciprocal(rec, pv[:, :, d])
            o16 = fix_pool.tile([128, nsb, d], f16, tag=f"o16{c}")
            nc.vector.tensor_tensor(
                out=o16,
                in0=pv[:, :, 0:d],
                in1=rec.unsqueeze(2).broadcast_to([128, nsb, d]),
                op=mybir.AluOpType.mult,
            )
            base = w * WIN + c * SQ
            nc.sync.dma_start(
                out=o[h, base : base + SQ, :].rearrange("(q p) d -> p q d", p=128),
                in_=o16,
            )

        windows = [(h, w) for h in range(heads) for w in range(nwin)]
        for h, w in windows:
            if w == 0:
                load_head(h)
            win_pts = []
            pvs = [
                po_pool.tile(
                    [128, nsb, d + 1], f32, tag="pv", name=f"pv{c}_{h}_{w}"
                )
                for c in (0, 1)
            ]
            gen = pv_slices(h, w, win_pts, pvs)
            for gi in range(ngrp):
                win_pts.append(emit_score_group(h, w, gi))
                if gi >= PV_LAG:
                    next(gen)
            for _ in gen:
                pass
            for c in (0, 1):
                fixup(h, w, pvs, c)


def verify_ldweights(nc):
    """Walk the final instruction order and assert every matmul's stationary
    operand matches the weights loaded by the preceding InstLdweights."""
    for f in nc.m.functions:
        for bb in f.blocks:
            last_w = None
            for ins in bb.instructions:
                if isinstance(ins, mybir.InstLdweights):
                    last_w = str(ins.ins[0])
                elif isinstance(ins, mybir.InstMatmult):
                    if ins.is_transpose:
                        last_w = None
                    else:
                        w = str(ins.ins[1])
                        assert last_w == w, (
                            f"{ins.name}: stationary mismatch\n"
                            f"loaded: {last_w}\nneeds:  {w}"
                        )


def build_program(heads=HPC, s=S, d=D):
    nc = bacc.Bacc(
        "TRN2", target_bir_lowering=False, debug=False, num_devices=N_CORES
    )
    qt = nc.dram_tensor("qt", [heads, d, s], mybir.dt.float16, kind="ExternalInput").ap()
    kt = nc.dram_tensor("kt", [heads, d, s], mybir.dt.float16, kind="ExternalInput").ap()
    va = nc.dram_tensor(
        "va", [heads, s, d + 1], mybir.dt.float16, kind="ExternalInput"
    ).ap()
    o = nc.dram_tensor("o", [heads, s, d], mybir.dt.float16, kind="ExternalOutput").ap()
    with tile.TileContext(nc) as tc:
        attention_body(tc, qt, kt, va, o, heads, s, d)
    nc.compile()
    verify_ldweights(nc)
    return nc


def prep_core_inputs(Qc, Kc, Vc):
    """Host-side layout prep for one core's [heads, s, d] fp16 slices."""
    heads, s, d = Qc.shape
    qt = np.ascontiguousarray(Qc.transpose(0, 2, 1))
    k4 = Kc.reshape(heads, 128, s // 128, d)
    kt = np.ascontiguousarray(k4.transpose(0, 3, 2, 1)).reshape(heads, d, s)
    va = np.concatenate([Vc, np.ones((heads, s, 1), np.float16)], axis=2)
    return {"qt": qt, "kt": kt, "va": np.ascontiguousarray(va)}


_cache = {}


def kernel(Q, K, V):
    Q = np.asarray(Q, dtype=np.float16)
    K = np.asarray(K, dtype=np.float16)
    V = np.asarray(V, dtype=np.float16)
    b, h, s, d = Q.shape
    assert (b, h, s, d) == (B, H, S, D)

    if "nc" not in _cache:
        _cache["nc"] = build_program()
    nc = _cache["nc"]

    Qf = Q.reshape(b * h, s, d)
    Kf = K.reshape(b * h, s, d)
    Vf = V.reshape(b * h, s, d)
    in_maps = [
        prep_core_inputs(
            Qf[c * HPC : (c + 1) * HPC],
            Kf[c * HPC : (c + 1) * HPC],
            Vf[c * HPC : (c + 1) * HPC],
        )
        for c in range(N_CORES)
    ]
    res = run_bass_kernel_spmd(nc, in_maps, core_ids=list(range(N_CORES)))
    outs = [res.results[c]["o"] for c in range(N_CORES)]
    return np.concatenate(outs, axis=0).reshape(b, h, s, d)


# revision 21
# speedup vs baseline: 1.1706x; 1.1706x over previous
"""Multi-head attention (B=2, H=16, S=4096, D=64, fp16) on 8 TRN2 NeuronCores.

Sharding: the 32 (b, h) head-slices are split 4-per-core (data/head
parallel, no cross-core communication). Each core runs a flash-attention
style kernel over its 4 heads.

Per-head algorithm (transposed-scores layout, no on-device transposes):
  - Host pre-lays-out inputs: QT[d, s] = Q^T, KTp[d, j*128+p] = K[p*32+j, d]
    (a t-permutation that makes the V load contiguous), and VA = [V | 1]
    (ones column => the PV matmul also produces the softmax denominator).
    QT/KT are loaded twice (partitions 0-63 and 64-127) so score matmuls can
    be row-packed onto both halves of the PE array (concurrent execution).
  - scores^T tile [t=128, s=512] = KTp_tile.T @ QT_tile   (PE, K=64)
  - P^T = exp(scale * scores^T)  fp32->fp16. The exp work is split between
    two engines, both reading the PSUM scores directly:
      * ACT groups: nc.scalar.activation(Exp)             (1 elem/cyc/lane)
      * DVE groups: one fused tensor_scalar computing
           i16 = round(score * (scale*log2e*2^10) + (15*2^10 - C))
        whose int16 bits, reinterpreted as fp16, are 2^(score*scale*log2e)
        with mantissa-linear (Schraudolph) interpolation: ~2% RMS rel err
        on those tiles, which dilutes to ~1e-2 on the final output --
        inside the 2e-2 accuracy budget.
  - PV (swapped operands): for each 128-wide query block sb,
        out[sb, 0:65] += P^T[t, sb].T @ VA[t, 0:65]
    i.e. the P^T tile is the *stationary* operand (128x128 fp16 loads get
    FWL's 2x weight-load rate) and VA streams only 65 columns. Measured
    45.5 ns/tile vs 66 ns/tile for the N=512 streaming formulation, and the
    accumulator is already in [s, d] layout -- no transposes needed.
    Column 64 of the accumulator is the softmax denominator.
  - fixup per 512-wide chunk: reciprocal of col 64, broadcast-multiply
    cols 0-63 (DVE, straight from PSUM), DMA out [s, d].

The emission runs a one-window software pipeline: while window w's scores
stream through PE->{ACT,DVE}, the PV matmuls consume window w-1's probs
(already in SBUF). Softmax skips max-subtraction: scores ~ N(0,1) after
scaling (measured |score*scale| < 6), so fp32/fp16 exp are safe.
"""

from contextlib import ExitStack

import numpy as np

import concourse.bass as bass
import concourse.tile as tile
from concourse import bacc, mybir
from concourse.bass_utils import run_bass_kernel_spmd

B, H, S, D = 2, 16, 4096, 64
N_CORES = 8
HPC = (B * H) // N_CORES  # heads per core
SCALE = float(D) ** -0.5
SQ = 512  # s-chunk width (one PSUM bank of fp32)
G = 2  # t-tiles (PSUM banks) per exp group
WIN = 2 * SQ  # s-window: scores/exp pipelining granularity

ROWPACK_SCORES = True  # tile_position row-packed scores matmuls
WARMUP = True  # HAM warmup matmul block

# Schraudolph exp2-in-fp16-bits constants (DVE exp path)
LOG2E = 1.4426950408889634
EXP_A = float(SCALE * LOG2E * 1024.0)
EXP_C = 60.0  # centering constant, tuned for min output L2 error
EXP_B = float(15 * 1024 - EXP_C)
# exp engine split: chunk 0 -> ACT, chunk 1 -> DVE (independent pacing
# lanes); group indices listed here run chunk 1 on ACT too (balance knob).
ACT_EXTRA = frozenset({15})


def attention_body(tc, qt, kt, va, o, heads, s, d):
    """Emit the per-core attention program.

    qt: [heads, d, s] fp16   Q^T per head
    kt: [heads, d, s] fp16   K^T per head, t-permuted (col j*128+p = row p*(s//128)+j)
    va: [heads, s, d+1] fp16 V with ones column, partition-major
    o:  [heads, s, d] fp16   output
    """
    nc = tc.nc
    f32 = mybir.dt.float32
    f16 = mybir.dt.float16
    i16 = mybir.dt.int16
    nt = s // 128  # number of 128-row t tiles
    nwin = s // WIN  # s windows per head
    nsb = SQ // 128  # 128-wide query blocks per chunk

    groups = []
    t0 = 0
    while t0 < nt:
        gs = min(G, nt - t0)
        groups.append((t0, gs))
        t0 += gs

    with ExitStack() as ctx:
        qk_pool = ctx.enter_context(tc.tile_pool(name="qk", bufs=2))
        v_pool = ctx.enter_context(tc.tile_pool(name="v", bufs=2))
        # probs live from their exp (window w) until consumed by PV during
        # window w+1: ~2 windows of groups in flight.
        # probs live ~PV_LAG+1 groups (exp at group g, consumed by the PV
        # slice interleaved after group g+PV_LAG): small rotating pool.
        p_pool = ctx.enter_context(tc.tile_pool(name="p", bufs=16))
        ps_pool = ctx.enter_context(tc.tile_pool(name="ps", bufs=3, space="PSUM"))
        po_pool = ctx.enter_context(tc.tile_pool(name="po", bufs=2, space="PSUM"))
        fix_pool = ctx.enter_context(tc.tile_pool(name="fix", bufs=2))
        const_pool = ctx.enter_context(tc.tile_pool(name="const", bufs=1))

        if WARMUP:
            # ~16 back-to-back matmuls trip the HAM activity window early so
            # the PE runs at 2.4 GHz instead of staying clock-gated at 1.2.
            warm_src = const_pool.tile([d + 1, SQ], f16)
            nc.vector.memset(warm_src, 1.0)
            warm_w = const_pool.tile([d + 1, d + 1], f16)
            nc.vector.memset(warm_w, 1.0)
            warm_ps = ps_pool.tile([128, G, SQ], f32, tag="ps")
            for i in range(16):
                nc.tensor.matmul(
                    warm_ps[: d + 1, 0, :],
                    lhsT=warm_w,
                    rhs=warm_src,
                    start=True,
                    stop=True,
                )

        # Per-head SBUF tiles, fetched lazily at head boundaries.
        head_tiles = {}

        def load_head(h):
            # Chunked loads ordered by first use so the first window's scores
            # only wait on the leading slices (Tile tracks byte-range deps).
            nck = 4
            cs = s // nck
            qt_sb = qk_pool.tile([128 if ROWPACK_SCORES else 64, s], f16, tag="qt")
            kt_sb = qk_pool.tile([128 if ROWPACK_SCORES else 64, s], f16, tag="kt")
            va_sb = v_pool.tile([128, nt, d + 1], f16, tag="va")
            va_src = va[h].rearrange("(p i) e -> p i e", p=128)
            rows = [0, 64] if ROWPACK_SCORES else [0]
            ick = nt // nck

            def kt_chunk(k):
                sl = slice(k * cs, (k + 1) * cs)
                for rp in rows:
                    nc.sync.dma_start(out=kt_sb[rp : rp + 64, sl], in_=kt[h][:, sl])

            def qt_chunk(k):
                sl = slice(k * cs, (k + 1) * cs)
                for rp in rows:
                    nc.sync.dma_start(out=qt_sb[rp : rp + 64, sl], in_=qt[h][:, sl])

            # kt chunk 0 + qt chunk 0 unblock the first window's scores; va is
            # first needed a window later; qt tails are needed last.
            kt_chunk(0)
            qt_chunk(0)
            for k in range(1, nck):
                kt_chunk(k)
            for k in range(nck):
                nc.sync.dma_start(
                    out=va_sb[:, k * ick : (k + 1) * ick, :],
                    in_=va_src[:, k * ick : (k + 1) * ick, :],
                )
            for k in range(1, nck):
                qt_chunk(k)
            head_tiles[h] = (qt_sb, kt_sb, va_sb)

        score_mm_count = [0]  # global parity so consecutive mms always row-pair

        def emit_score_group(h, w, gi):
            """Scores (both chunks, fully row-paired) + exp for group gi."""
            qt_sb, kt_sb, _ = head_tiles[h]
            w0 = w * WIN
            t0, gs = groups[gi]
            pss = []
            # all 2*gs score matmuls adjacent on the PE queue with alternating
            # row parity -> every consecutive pair runs concurrently
            for c in (0, 1):
                ps = ps_pool.tile([128, G, SQ], f32, tag="ps", name=f"ps{c}")
                pss.append(ps)
            for c in (0, 1):
                for g in range(gs):
                    t = t0 + g
                    rp = 64 * (score_mm_count[0] % 2) if ROWPACK_SCORES else 0
                    score_mm_count[0] += 1
                    nc.tensor.matmul(
                        pss[c][:, g, :],
                        lhsT=kt_sb[rp : rp + 64, t * 128 : (t + 1) * 128],
                        rhs=qt_sb[rp : rp + 64, w0 + c * SQ : w0 + (c + 1) * SQ],
                        start=True,
                        stop=True,
                        tile_position=(rp, 0) if ROWPACK_SCORES else None,
                    )
            pts = []
            for c in (0, 1):
                pt = p_pool.tile([128, G, SQ], f16, tag="pt", name=f"pt{c}")
                if c == 1 and gi not in ACT_EXTRA:
                    nc.vector.tensor_scalar(
                        pt[:, :gs, :].bitcast(i16),
                        pss[c][:, :gs, :],
                        EXP_A,
                        EXP_B,
                        mybir.AluOpType.mult,
                        mybir.AluOpType.add,
                    )
                else:
                    nc.scalar.activation(
                        pt[:, :gs, :],
                        pss[c][:, :gs, :],
                        mybir.ActivationFunctionType.Exp,
                        scale=SCALE,
                    )
                pts.append(pt)
            return pts

        ngrp = len(groups)
        PV_LAG = 4  # groups between a group's exp and its PV slice

        def pv_slices(h, w, win_pts, pvs):
            """Yields once per group after emitting that group's PV matmuls.

            The t-accumulation is ordered by group so PV of window w runs
            *inside* window w, lagged PV_LAG groups behind the exps: the
            software pipeline is ~2 groups deep instead of a full window
            (cheap fill/drain, small probs pool), and the PE interleaves
            scores + PV continuously (HAM stays at full clock)."""
            _, _, va_sb = head_tiles[h]
            for g in range(ngrp):
                t0, gs = groups[g]
                pts = win_pts[g]  # filled by the caller before this slice runs
                for c in (0, 1):
                    for sb in range(nsb):
                        for gg in range(gs):
                            t = t0 + gg
                            # start=True clears has_written for the WHOLE
                            # bank, so only the bank's first mm may set it;
                            # the other sb-chains' first writes land on
                            # cleared bits and overwrite (then accumulate).
                            nc.tensor.matmul(
                                pvs[c][:, sb, :],
                                lhsT=pts[c][:, gg, sb * 128 : (sb + 1) * 128],
                                rhs=va_sb[:, t, :],
                                start=(t == 0 and sb == 0),
                                stop=(t == nt - 1 and sb == nsb - 1),
                                skip_group_check=True,
                            )
                yield

        def fixup(h, w, pvs, c):
            pv = pvs[c]
            rec = fix_pool.tile([128, nsb], f32, tag=f"rec{c}")
            nc.vector.reciprocal(rec, pv[:, :, d])
            o16 = fix_pool.tile([128, nsb, d], f16, tag=f"o16{c}")
            nc.vector.tensor_tensor(
                out=o16,
                in0=pv[:, :, 0:d],
                in1=rec.unsqueeze(2).broadcast_to([128, nsb, d]),
                op=mybir.AluOpType.mult,
            )
            base = w * WIN + c * SQ
            nc.sync.dma_start(
                out=o[h, base : base + SQ, :].rearrange("(q p) d -> p q d", p=128),
                in_=o16,
            )

        windows = [(h, w) for h in range(heads) for w in range(nwin)]
        for h, w in windows:
            if w == 0:
                load_head(h)
            win_pts = []
            pvs = [
                po_pool.tile(
                    [128, nsb, d + 1], f32, tag="pv", name=f"pv{c}_{h}_{w}"
                )
                for c in (0, 1)
            ]
            gen = pv_slices(h, w, win_pts, pvs)
            for gi in range(ngrp):
                win_pts.append(emit_score_group(h, w, gi))
                if gi >= PV_LAG:
                    next(gen)
            for _ in gen:
                pass
            for c in (0, 1):
                fixup(h, w, pvs, c)


def verify_ldweights(nc):
    """Walk the final instruction order and assert every matmul's stationary
    operand matches the weights loaded by the preceding InstLdweights."""
    for f in nc.m.functions:
        for bb in f.blocks:
            last_w = None
            for ins in bb.instructions:
                if isinstance(ins, mybir.InstLdweights):
                    last_w = str(ins.ins[0])
                elif isinstance(ins, mybir.InstMatmult):
                    if ins.is_transpose:
                        last_w = None
                    else:
                        w = str(ins.ins[1])
                        assert last_w == w, (
                            f"{ins.name}: stationary mismatch\n"
                            f"loaded: {last_w}\nneeds:  {w}"
                        )


def build_program(heads=HPC, s=S, d=D):
    nc = bacc.Bacc(
        "TRN2", target_bir_lowering=False, debug=False, num_devices=N_CORES
    )
    qt = nc.dram_tensor("qt", [heads, d, s], mybir.dt.float16, kind="ExternalInput").ap()
    kt = nc.dram_tensor("kt", [heads, d, s], mybir.dt.float16, kind="ExternalInput").ap()
    va = nc.dram_tensor(
        "va", [heads, s, d + 1], mybir.dt.float16, kind="ExternalInput"
    ).ap()
    o = nc.dram_tensor("o", [heads, s, d], mybir.dt.float16, kind="ExternalOutput").ap()
    with tile.TileContext(nc) as tc:
        attention_body(tc, qt, kt, va, o, heads, s, d)
    nc.compile()
    verify_ldweights(nc)
    return nc


def prep_core_inputs(Qc, Kc, Vc):
    """Host-side layout prep for one core's [heads, s, d] fp16 slices."""
    heads, s, d = Qc.shape
    qt = np.ascontiguousarray(Qc.transpose(0, 2, 1))
    k4 = Kc.reshape(heads, 128, s // 128, d)
    kt = np.ascontiguousarray(k4.transpose(0, 3, 2, 1)).reshape(heads, d, s)
    va = np.concatenate([Vc, np.ones((heads, s, 1), np.float16)], axis=2)
    return {"qt": qt, "kt": kt, "va": np.ascontiguousarray(va)}


_cache = {}


def kernel(Q, K, V):
    Q = np.asarray(Q, dtype=np.float16)
    K = np.asarray(K, dtype=np.float16)
    V = np.asarray(V, dtype=np.float16)
    b, h, s, d = Q.shape
    assert (b, h, s, d) == (B, H, S, D)

    if "nc" not in _cache:
        _cache["nc"] = build_program()
    nc = _cache["nc"]

    Qf = Q.reshape(b * h, s, d)
    Kf = K.reshape(b * h, s, d)
    Vf = V.reshape(b * h, s, d)
    in_maps = [
        prep_core_inputs(
            Qf[c * HPC : (c + 1) * HPC],
            Kf[c * HPC : (c + 1) * HPC],
            Vf[c * HPC : (c + 1) * HPC],
        )
        for c in range(N_CORES)
    ]
    res = run_bass_kernel_spmd(nc, in_maps, core_ids=list(range(N_CORES)))
    outs = [res.results[c]["o"] for c in range(N_CORES)]
    return np.concatenate(outs, axis=0).reshape(b, h, s, d)
